# revision 36
# baseline (speedup 1.0000x reference)
"""Dual-path multi-head attention on 8 trn2 NeuronCores.

Sharding: core c = (path p=c//4, batch b=c%4). Each core runs the full
pipeline for one path and one batch element: 3 input projections, 16-head
attention (S=1024, dh=64), output projection. No collectives.

Path 2 cross-wiring (q2 from k; k2,v2 from q) is handled purely by host-side
input routing - every core runs the identical SPMD program.

Key speed tricks vs a plain bf16 pipeline:
- QKV projections run as fp8e4m3 DoubleRow matmuls (2 contraction rows per
  PE column pass) with a 3-pass residual decomposition
  W.x ~= Wh.xh + Wh.xr + Wr.xh (h = fp8(v), r = fp8(v - h)), all packed on
  the host. W is pre-scaled by 32 so its values sit in e4m3's normal range;
  the 32*32 factor is folded into the softmax exp scale, and 1/32 into the
  host-packed Wc. Same accuracy as bf16 at half the PE time.
- PV runs transposed: stationary = probs chunk [128k, 128q], moving =
  v1e head slot [128k, 65] -> psum [128q, 65]. Out free size 65 instead of
  512 halves PE rows; the softmax denominator rides along as a ones column
  (col 64), and normalization becomes a per-partition tensor_scalar multiply
  (no partition broadcast needed).
- The resulting [q, d] attention output is PE-transposed (128x128 blocks)
  back to [d, q] for the output projection.
- Scores psum uses a 4-bank X tile (2 key chunks -> one 2048-wide exp) plus
  a 2-bank Y tile that alternates between single-chunk scores and the next
  block's Q/K projection psum, giving 3x2048+2x1024 exp batching per head
  while fitting the 8 psum banks alongside the PV accumulators.

Emission order software-pipelines: head h emits its scores/exp interleaved
with PV+normalize of head h-1 and one Q/K projection block, so PE and Act
stay concurrently busy through the 16-head phase.
"""

import numpy as np
import ml_dtypes

B, S, D, H, DH = 4, 1024, 1024, 16, 64
NB = D // 128   # 8 partition blocks
NC2 = D // 256  # 4 pair-chunks for DoubleRow
HW = 65         # head slot width in v1e (64 data + 1 ones col)

_compiled = None


def _build():
    import concourse.bass as bass
    import concourse.mybir as mybir
    import concourse.tile as tile
    from concourse import bacc, masks

    dt = mybir.dt
    f32, bf16, f32r, fp8 = dt.float32, dt.bfloat16, dt.float32r, dt.float8e4
    DR = mybir.MatmulPerfMode.DoubleRow
    ExpF = mybir.ActivationFunctionType.Exp

    nc = bacc.Bacc("TRN2", target_bir_lowering=False, debug=False)

    xqh_d = nc.dram_tensor("xqh", [128, NC2, 2, S], fp8, kind="ExternalInput")
    xqr_d = nc.dram_tensor("xqr", [128, NC2, 2, S], fp8, kind="ExternalInput")
    xkh_d = nc.dram_tensor("xkh", [128, NC2, 2, S], fp8, kind="ExternalInput")
    xkr_d = nc.dram_tensor("xkr", [128, NC2, 2, S], fp8, kind="ExternalInput")
    xvh_d = nc.dram_tensor("xvh", [128, NC2, 2, S], fp8, kind="ExternalInput")
    xvr_d = nc.dram_tensor("xvr", [128, NC2, 2, S], fp8, kind="ExternalInput")
    wqh_d = nc.dram_tensor("wqh", [128, NB, NC2, 2, 128], fp8, kind="ExternalInput")
    wqr_d = nc.dram_tensor("wqr", [128, NB, NC2, 2, 128], fp8, kind="ExternalInput")
    wkh_d = nc.dram_tensor("wkh", [128, NB, NC2, 2, 128], fp8, kind="ExternalInput")
    wkr_d = nc.dram_tensor("wkr", [128, NB, NC2, 2, 128], fp8, kind="ExternalInput")
    wvh_d = nc.dram_tensor("wvh", [128, NC2, 2, D], fp8, kind="ExternalInput")
    wvr_d = nc.dram_tensor("wvr", [128, NC2, 2, D], fp8, kind="ExternalInput")
    wcl_d = nc.dram_tensor("wcl", [128, NB // 2, NB, 128], bf16, kind="ExternalInput")
    wch_d = nc.dram_tensor("wch", [128, NB // 2, NB, 128], bf16, kind="ExternalInput")
    bqkc_d = nc.dram_tensor("bqkc", [128, 3, NB], f32, kind="ExternalInput")
    bvB_d = nc.dram_tensor("bvB", [128, D], bf16, kind="ExternalInput")
    out_d = nc.dram_tensor("outT", [D, S], bf16, kind="ExternalOutput")

    ESCALE = 0.125 / 1024.0  # 1/sqrt(dh) softmax scale / (32*32 weight scale)

    with tile.TileContext(nc) as tc:
        with tc.tile_pool(name="xin", bufs=1) as xip, \
             tc.tile_pool(name="b8", bufs=2) as b8p, \
             tc.tile_pool(name="wv", bufs=1) as wvp, \
             tc.tile_pool(name="pt", bufs=3) as ptp, \
             tc.tile_pool(name="wqk", bufs=2) as wkp, \
             tc.tile_pool(name="qk", bufs=2) as qkp, \
             tc.tile_pool(name="pers", bufs=1) as prp, \
             tc.tile_pool(name="aqp", bufs=2) as aqp, \
             tc.tile_pool(name="rc", bufs=4) as rcp, \
             tc.tile_pool(name="ost", bufs=4) as ostp, \
             tc.tile_pool(name="pxa", bufs=1, space="PSUM") as pxap, \
             tc.tile_pool(name="pxb", bufs=1, space="PSUM") as pxbp, \
             tc.tile_pool(name="pj", bufs=1, space="PSUM") as pjp, \
             tc.tile_pool(name="pv", bufs=1, space="PSUM") as pvp:

            # ---- input DMAs. HWDGE (sync+scalar queues) serializes dispatch
            # at ~650ns/DMA; gpsimd (SWDGE) costs ~1us Pool trigger per DMA
            # but transfers overlap. Q/K-projection inputs lead on both.
            def wblk_load(w_d, m, nm, engine):
                wb = wkp.tile([128, NC2, 2, 128], fp8, tag=nm, name=f"{nm}{m}")
                engine.dma_start(out=wb[:, :, :, :], in_=w_d.ap()[:, m, :, :, :])
                return wb

            xqh_t = xip.tile([128, NC2, 2, S], fp8, tag="xqh")
            nc.gpsimd.dma_start(out=xqh_t[:, :, :, :], in_=xqh_d.ap())
            bqkc_t = xip.tile([128, 3, NB], f32, tag="bqkc")
            nc.sync.dma_start(out=bqkc_t[:, :, :], in_=bqkc_d.ap())
            bq_t, bk_t, bc_t = bqkc_t[:, 0, :], bqkc_t[:, 1, :], bqkc_t[:, 2, :]
            xqr_t = xip.tile([128, NC2, 2, S], fp8, tag="xqr")
            nc.sync.dma_start(out=xqr_t[:, :, :, :], in_=xqr_d.ap())
            wq0h = wblk_load(wqh_d, 0, "wqh", nc.gpsimd)
            wq0r = wblk_load(wqr_d, 0, "wqr", nc.gpsimd)
            wk0h = wblk_load(wkh_d, 0, "wkh", nc.sync)
            wk0r = wblk_load(wkr_d, 0, "wkr", nc.sync)

            xkh_t = xip.tile([128, NC2, 2, S], fp8, tag="xkh")
            nc.sync.dma_start(out=xkh_t[:, :, :, :], in_=xkh_d.ap())
            xkr_t = xip.tile([128, NC2, 2, S], fp8, tag="xkr")
            nc.sync.dma_start(out=xkr_t[:, :, :, :], in_=xkr_d.ap())

            xvh_t = b8p.tile([128, NC2, 2, S], fp8, tag="b8", name="xvh_t")
            nc.sync.dma_start(out=xvh_t[:, :, :, :], in_=xvh_d.ap())
            wvh_t = wvp.tile([128, NC2, 2, D], fp8, tag="wvh")
            nc.gpsimd.dma_start(out=wvh_t[:, :, :, :], in_=wvh_d.ap())
            wvr_t = wvp.tile([128, NC2, 2, D], fp8, tag="wvr")
            nc.sync.dma_start(out=wvr_t[:, :, :, :], in_=wvr_d.ap())
            xvr_t = b8p.tile([128, NC2, 2, S], fp8, tag="b8", name="xvr_t")
            nc.gpsimd.dma_start(out=xvr_t[:, :, :, :], in_=xvr_d.ap())
            bvB_t = xip.tile([128, D], bf16, tag="bv")
            nc.sync.dma_start(out=bvB_t[:, :], in_=bvB_d.ap())

            ident = xip.tile([128, 128], bf16, tag="id")
            masks.make_identity(nc, ident[:, :])

            # persistent tiles
            v1e = prp.tile([128, NB, H * HW], bf16)
            a1 = prp.tile([128, NB, S], bf16, tag="a1")
            ones_ap = v1e[:, :, :].rearrange("p n (h x) -> p n h x", x=HW)[:, :, :, 64]
            nc.vector.memset(ones_ap, 1.0)

            # ---- half-granular projection emitters (fp8 DoubleRow 3-pass) ----
            def vproj_half_mm(ps_view, n2, half):
                first = True
                for xa, wa in ((xvh_t, wvh_t), (xvr_t, wvh_t), (xvh_t, wvr_t)):
                    for c in range(NC2):
                        nc.tensor.matmul(
                            ps_view,
                            xa[:, c, :, n2 * 128:(n2 + 1) * 128],
                            wa[:, c, :, half * 512:(half + 1) * 512],
                            start=first, stop=(wa is wvr_t and c == NC2 - 1),
                            perf_mode=DR,
                        )
                        first = False

            def vproj_drain(ps_view, n2, half):
                dst = v1e[:, n2, :].rearrange(
                    "p (h x) -> p h x", x=HW)[:, half * 8:(half + 1) * 8, 0:64]
                nc.vector.tensor_add(
                    dst,
                    ps_view.rearrange("p (h x) -> p h x", x=64),
                    bvB_t[:, half * 512:(half + 1) * 512].rearrange(
                        "p (h x) -> p h x", x=64))

            def vproj_block(n2, pool):
                ps = pool.tile([128, 2, 512], f32, tag="s", name=f"vps{n2}")
                for half in range(2):
                    vproj_half_mm(ps[:, half, :], n2, half)
                    vproj_drain(ps[:, half, :], n2, half)

            def vproj_half_pj(n2, half):
                ps = pjp.tile([128, 512], f32, tag="s", name=f"vpj{n2}{half}")
                vproj_half_mm(ps[:, :], n2, half)
                vproj_drain(ps[:, :], n2, half)

            def proj_half(wbh, wbr, xh, xr, b_t, ob, m, half):
                ps = pjp.tile([128, 512], f32, tag="s", name=f"pps{m}{half}")
                first = True
                for wa, xa in ((wbh, xh), (wbr, xh), (wbh, xr)):
                    for c in range(NC2):
                        nc.tensor.matmul(
                            ps[:, :], wa[:, c, :, :],
                            xa[:, c, :, half * 512:(half + 1) * 512],
                            start=first, stop=(xa is xr and c == NC2 - 1),
                            perf_mode=DR,
                        )
                        first = False
                nc.vector.tensor_scalar_add(
                    ob[:, half * 512:(half + 1) * 512], ps[:, :], b_t[:, m:m + 1])

            def new_qk(m, which):
                return qkp.tile([128, S], f32r, tag=which, name=f"{which}_{m}")

            # ---- prologue: QK block 0 first (heads start early), then V ----
            q1b = new_qk(0, "q1")
            k1b = new_qk(0, "k1")
            proj_half(wq0h, wq0r, xqh_t, xqr_t, bq_t, q1b, 0, 0)
            proj_half(wk0h, wk0r, xkh_t, xkr_t, bk_t, k1b, 0, 0)
            proj_half(wq0h, wq0r, xqh_t, xqr_t, bq_t, q1b, 0, 1)
            proj_half(wk0h, wk0r, xkh_t, xkr_t, bk_t, k1b, 0, 1)
            for n2 in range(6):
                vproj_block(n2, pxap if n2 % 2 == 0 else pxbp)
            # V blocks 6,7 run through the pj slot during head 0

            # ---- attention phase ----
            state = {}  # deferred work for head h-1

            def scores_group(pt, q1b, k1b, po, g):
                """Half-chunks 3g..3g+2 (g=5: just one) into slot A/B, one
                1536-wide (or 512) exp. Half-chunk j covers scores chunk
                n=j//2, query half j%2 -> pt flat cols [j*512, j*512+512)."""
                pool = pxap if g % 2 == 0 else pxbp
                js = list(range(3 * g, min(3 * g + 3, 16)))
                xs = pool.tile([128, 3, 512], f32, tag="s", name=f"sg{g}")
                for pos, j in enumerate(js):
                    n, half = j // 2, j % 2
                    nc.tensor.matmul(
                        xs[:, pos, :],
                        k1b[po:po + 64, n * 128:(n + 1) * 128],
                        q1b[po:po + 64, half * 512:(half + 1) * 512],
                        start=True, stop=True,
                    )
                flat = pt[:, :, :].rearrange("p a b -> p (a b)")
                np_ = len(js)
                nc.scalar.activation(
                    out=flat[:, 3 * g * 512:(3 * g + np_) * 512],
                    in_=xs[:, 0:np_, :].rearrange("p a b -> p (a b)"),
                    func=ExpF, scale=ESCALE)

            class PVState:
                """PV + normalize for one head; emitted during the next head."""

                def __init__(self, h, pt, aq):
                    self.h, self.pt, self.aq = h, pt, aq
                    self.ps = {}

                def pv(self, tag):
                    g = 0 if tag == "pva" else 1
                    ps = pvp.tile([128, 4, HW], f32, tag="pv",
                                  name=f"pv{self.h}{tag}")
                    self.ps[tag] = ps
                    for qc in range(g * 4, g * 4 + 4):
                        for n in range(NB):
                            nc.tensor.matmul(
                                ps[:, qc - g * 4, :],
                                self.pt[:, n, qc * 128:(qc + 1) * 128],
                                v1e[:, n, self.h * HW:(self.h + 1) * HW],
                                start=(n == 0), stop=(n == NB - 1),
                            )

                def norm(self, tag):
                    g = 0 if tag == "pva" else 1
                    po = (self.h % 2) * 64
                    ps = self.ps[tag]
                    rc = rcp.tile([128, 4, 1], f32, tag="rc",
                                  name=f"rc{self.h}{g}")
                    nc.vector.reciprocal(rc[:, :, :], ps[:, :, 64:65])
                    nc.vector.tensor_mul(
                        self.aq[:, g * 4:(g + 1) * 4, po:po + 64],
                        ps[:, :, 0:64],
                        rc[:, :, :].to_broadcast((128, 4, 64)))

            def transpose_pair(m, aq):
                tp = pvp.tile([128, NB, 128], bf16, tag="pv", name=f"tp{m}")
                for qc in range(NB):
                    nc.tensor.transpose(tp[:, qc, :], aq[:, qc, :], ident[:, :])
                nc.vector.tensor_copy(
                    a1[:, m, :], tp[:, :, :].rearrange("p a b -> p (a b)"))

            qk_tiles = {0: [q1b, k1b]}
            wtiles = {}
            prev = None
            aq_cur = None
            wcl_t = wch_t = None

            # out-projection split: partials (pairs 0-5) pre-run during the
            # proj-free endgame heads; finals (pairs 6,7 + partial add) at the
            # end. Bias is applied in the partial drain.
            part_sb = prp.tile([128, NB, S], bf16, tag="part")
            op_parts = [(m2, hf) for m2 in range(NB) for hf in range(2)]

            def wct_slice(m2, n):
                wct = wcl_t if m2 < NB // 2 else wch_t
                return wct[:, m2 % (NB // 2), n, :]

            def op_partial():
                m2, hf = op_parts.pop(0)
                ps = pjp.tile([128, 512], f32, tag="s", name=f"op{m2}{hf}")
                for n in range(6):
                    nc.tensor.matmul(
                        ps[:, :], wct_slice(m2, n),
                        a1[:, n, hf * 512:(hf + 1) * 512],
                        start=(n == 0), stop=(n == 5),
                    )
                nc.vector.tensor_scalar_add(
                    part_sb[:, m2, hf * 512:(hf + 1) * 512], ps[:, :],
                    bc_t[:, m2:m2 + 1])
            for h in range(H):
                m = h // 2
                po = (h % 2) * 64
                q1b, k1b = qk_tiles[m]
                pt = ptp.tile([128, NB, S], bf16, tag="pt", name=f"pt{h}")
                if h % 2 == 0:
                    aq_cur = aqp.tile([128, NB, 128], bf16, tag="aq", name=f"aq{m}")
                aq_h = aq_cur

                # pj-slot fillers for this head
                fl = []
                if h == 0:
                    wtiles[1] = (wblk_load(wqh_d, 1, "wqh", nc.scalar),
                                 wblk_load(wqr_d, 1, "wqr", nc.scalar),
                                 wblk_load(wkh_d, 1, "wkh", nc.scalar),
                                 wblk_load(wkr_d, 1, "wkr", nc.scalar))
                    fl = [(vproj_half_pj, (6, 0)), (vproj_half_pj, (6, 1)),
                          (vproj_half_pj, (7, 0)), (vproj_half_pj, (7, 1))]
                elif h == 1:
                    wqhn, wqrn, wkhn, wkrn = wtiles[1]
                    nq, nk = new_qk(1, "q1"), new_qk(1, "k1")
                    qk_tiles[1] = [nq, nk]
                    fl = [(proj_half, (wqhn, wqrn, xqh_t, xqr_t, bq_t, nq, 1, 0)),
                          (proj_half, (wqhn, wqrn, xqh_t, xqr_t, bq_t, nq, 1, 1)),
                          (proj_half, (wkhn, wkrn, xkh_t, xkr_t, bk_t, nk, 1, 0)),
                          (proj_half, (wkhn, wkrn, xkh_t, xkr_t, bk_t, nk, 1, 1))]
                elif m < NB - 1:
                    wqhn, wqrn, wkhn, wkrn = wtiles[m + 1]
                    if h % 2 == 0:
                        nq = new_qk(m + 1, "q1")
                        qk_tiles[m + 1] = [nq, None]
                        fl = [(proj_half, (wqhn, wqrn, xqh_t, xqr_t, bq_t, nq,
                                           m + 1, hf)) for hf in range(2)]
                    else:
                        nk = new_qk(m + 1, "k1")
                        qk_tiles[m + 1][1] = nk
                        fl = [(proj_half, (wkhn, wkrn, xkh_t, xkr_t, bk_t, nk,
                                           m + 1, hf)) for hf in range(2)]

                # stream weights two blocks ahead (odd heads)
                if h % 2 == 1 and m + 2 < NB:
                    wtiles[m + 2] = (wblk_load(wqh_d, m + 2, "wqh", nc.sync),
                                     wblk_load(wqr_d, m + 2, "wqr", nc.sync),
                                     wblk_load(wkh_d, m + 2, "wkh", nc.gpsimd),
                                     wblk_load(wkr_d, m + 2, "wkr", nc.gpsimd))
                if h == 1:
                    # wc loads reuse the xvh/xvr slots (b8 ring); V-proj done
                    wcl_t = b8p.tile([128, NB // 2, NB, 128], bf16, tag="b8",
                                     name="wcl_t")
                    nc.scalar.dma_start(out=wcl_t[:, :, :, :], in_=wcl_d.ap())
                    wch_t = b8p.tile([128, NB // 2, NB, 128], bf16, tag="b8",
                                     name="wch_t")
                    nc.scalar.dma_start(out=wch_t[:, :, :, :], in_=wch_d.ap())

                if h == 0:
                    # custom: V6/V7 pj fillers interleaved between groups
                    for g in range(6):
                        scores_group(pt, q1b, k1b, po, g)
                        if fl and g % 2 == 0:
                            f, a = fl.pop(0)
                            f(*a)
                        if fl and g == 5:
                            f, a = fl.pop(0)
                            f(*a)
                else:
                    scores_group(pt, q1b, k1b, po, 0)
                    scores_group(pt, q1b, k1b, po, 1)
                    # transposes of the pair finished two heads ago: emitted
                    # here so they execute once its norms drain, while PE has
                    # scores to chew on
                    if h >= 3 and h % 2 == 1:
                        transpose_pair(h // 2 - 1, aq_prev)
                    if prev is not None:
                        prev.pv("pva")
                        prev.norm("pva")
                    def filler():
                        if fl:
                            f, a = fl.pop(0)
                            f(*a)
                        elif h >= 14 and op_parts:
                            op_partial()

                    scores_group(pt, q1b, k1b, po, 2)
                    filler()
                    scores_group(pt, q1b, k1b, po, 3)
                    filler()
                    scores_group(pt, q1b, k1b, po, 4)
                    scores_group(pt, q1b, k1b, po, 5)
                    if prev is not None:
                        prev.pv("pvb")
                        prev.norm("pvb")
                    while fl:
                        f, a = fl.pop(0)
                        f(*a)
                    if h == 13 and op_parts:
                        op_partial()
                    if h >= 14:
                        for _ in range(2):
                            if op_parts:
                                op_partial()

                if h % 2 == 1:
                    aq_prev = aq_h
                prev = PVState(h, pt, aq_h)

            # finish head 15
            prev.pv("pva")
            prev.norm("pva")
            prev.pv("pvb")
            prev.norm("pvb")
            transpose_pair(7, prev.aq)

            # ---- out-projection finals: pairs 6,7 + partial add; remaining
            # partials (pj) interleave with finals (other rings) ----
            queues = [nc.sync, nc.gpsimd, nc.scalar]
            opools = [(pxap, "s"), (pxbp, "s"), (pvp, "pv")]
            for i, (m2, hf) in enumerate(
                    [(m2, hf) for m2 in range(NB) for hf in range(2)]):
                if op_parts:
                    op_partial()
                pool, ptag = opools[i % 3]
                ps = pool.tile([128, 512], f32, tag=ptag, name=f"of{m2}{hf}")
                for n in (6, 7):
                    nc.tensor.matmul(
                        ps[:, :], wct_slice(m2, n),
                        a1[:, n, hf * 512:(hf + 1) * 512],
                        start=(n == 6), stop=(n == 7),
                    )
                ot = ostp.tile([128, 512], bf16, tag="ost", name=f"ot{m2}{hf}")
                nc.vector.tensor_add(ot[:, :], ps[:, :],
                                     part_sb[:, m2, hf * 512:(hf + 1) * 512])
                queues[i % 3].dma_start(
                    out=out_d.ap()[m2 * 128:(m2 + 1) * 128,
                                   hf * 512:(hf + 1) * 512],
                    in_=ot[:, :])

    nc.compile()
    return nc


def _get_nc():
    global _compiled
    if _compiled is None:
        _compiled = _build()
    return _compiled


def _fp8_split(a):
    e4m3 = ml_dtypes.float8_e4m3
    h = np.ascontiguousarray(a).astype(e4m3)
    r = (a - h.astype(np.float32)).astype(e4m3)
    return h, r


def _make_in_maps(q, k, v, Wq, bq, Wk, bk, Wv, bv, Wq2, bq2, Wk2, bk2, Wv2, bv2,
                  Wc, bc, Wc2, bc2):
    bf16 = ml_dtypes.bfloat16

    def xpack(x):  # [s, d] -> 2x [128, c, 2, s] fp8 (d = (2c+j)*128+p)
        xt = np.asarray(x, np.float32).T.reshape(NC2, 2, 128, S).transpose(2, 0, 1, 3)
        return _fp8_split(np.ascontiguousarray(xt))

    def wqkpack(w):  # W[e,d]*32 -> 2x [128 p, m, c, 2, e'] fp8
        wt = (32.0 * np.asarray(w, np.float32)).reshape(
            NB, 128, NC2, 2, 128).transpose(4, 0, 2, 3, 1)
        return _fp8_split(np.ascontiguousarray(wt))

    def wvpack(w):  # Wv[e,d]*32 -> 2x [128 p, c, 2, e] fp8
        wt = (32.0 * np.asarray(w, np.float32)).T.reshape(
            NC2, 2, 128, D).transpose(2, 0, 1, 3)
        return _fp8_split(np.ascontiguousarray(wt))

    def wcpack(w):  # Wc[e,d]/32 -> [128 p, m, n, e'] bf16, split in m halves
        ws = (np.asarray(w, np.float32) / 32.0).reshape(
            NB, 128, NB, 128).transpose(3, 0, 2, 1)
        ws = np.ascontiguousarray(ws).astype(bf16)
        return ws[:, :NB // 2], ws[:, NB // 2:]

    def btile(b, scale):
        return np.ascontiguousarray(
            (scale * np.asarray(b, np.float32)).reshape(NB, 128).T)

    def brep(b, scale):
        return np.ascontiguousarray(np.broadcast_to(
            scale * np.asarray(b, np.float32), (128, D))).astype(bf16)

    paths = []
    for (Wq_, bq_, Wk_, bk_, Wv_, bv_, Wc_, bc_) in (
            (Wq, bq, Wk, bk, Wv, bv, Wc, bc),
            (Wq2, bq2, Wk2, bk2, Wv2, bv2, Wc2, bc2)):
        wqh, wqr = wqkpack(Wq_)
        wkh, wkr = wqkpack(Wk_)
        wvh, wvr = wvpack(Wv_)
        wcl, wch = wcpack(Wc_)
        bqkc = np.ascontiguousarray(np.stack(
            [btile(bq_, 32.0), btile(bk_, 32.0), btile(bc_, 1.0)], axis=1))
        paths.append(dict(
            wqh=wqh, wqr=wqr, wkh=wkh, wkr=wkr, wvh=wvh, wvr=wvr,
            wcl=wcl, wch=wch, bqkc=bqkc, bvB=brep(bv_, 32.0)))

    xs = {}
    for nmx, arr in (("q", q), ("k", k), ("v", v)):
        for b in range(B):
            xs[(nmx, b)] = xpack(arr[b])

    in_maps = []
    for c in range(8):
        p, b = c // 4, c % 4
        if p == 0:
            (xqh, xqr), (xkh, xkr), (xvh, xvr) = xs[("q", b)], xs[("k", b)], xs[("v", b)]
        else:
            # path 2: q2 from k; k2, v2 from q
            (xqh, xqr), (xkh, xkr), (xvh, xvr) = xs[("k", b)], xs[("q", b)], xs[("q", b)]
        in_maps.append(dict(paths[p], xqh=xqh, xqr=xqr, xkh=xkh, xkr=xkr,
                            xvh=xvh, xvr=xvr))
    return in_maps


def _run(in_maps, trace=False):
    from concourse.bass_utils import run_bass_kernel_spmd
    nc = _get_nc()
    return run_bass_kernel_spmd(nc, in_maps, core_ids=list(range(8)), trace=trace)


def kernel(**inputs):
    in_maps = _make_in_maps(**inputs)
    try:
        res = _run(in_maps)
    except Exception:
        # transient NRT_EXEC_UNIT_UNRECOVERABLE has been observed when a
        # prior process crashed mid-execution; one retry reloads the NEFF
        res = _run(in_maps)
    out1 = np.stack([res.results[b]["outT"].T for b in range(4)]).astype(np.float32)
    out2 = np.stack([res.results[4 + b]["outT"].T for b in range(4)]).astype(np.float32)
    return out1, out2


# revision 37
# speedup vs baseline: 1.0060x; 1.0060x over previous
"""Dual-path multi-head attention on 8 trn2 NeuronCores.

Sharding: core c = (path p=c//4, batch b=c%4). Each core runs the full
pipeline for one path and one batch element: 3 input projections, 16-head
attention (S=1024, dh=64), output projection. No collectives.

Path 2 cross-wiring (q2 from k; k2,v2 from q) is handled purely by host-side
input routing - every core runs the identical SPMD program.

Key speed tricks vs a plain bf16 pipeline:
- QKV projections run as fp8e4m3 DoubleRow matmuls (2 contraction rows per
  PE column pass) with a 3-pass residual decomposition
  W.x ~= Wh.xh + Wh.xr + Wr.xh (h = fp8(v), r = fp8(v - h)), all packed on
  the host. W is pre-scaled by 32 so its values sit in e4m3's normal range;
  the 32*32 factor is folded into the softmax exp scale, and 1/32 into the
  host-packed Wc. Same accuracy as bf16 at half the PE time.
- PV runs transposed: stationary = probs chunk [128k, 128q], moving =
  v1e head slot [128k, 65] -> psum [128q, 65]. Out free size 65 instead of
  512 halves PE rows; the softmax denominator rides along as a ones column
  (col 64), and normalization becomes a per-partition tensor_scalar multiply
  (no partition broadcast needed).
- The resulting [q, d] attention output is PE-transposed (128x128 blocks)
  back to [d, q] for the output projection.
- Scores psum uses a 4-bank X tile (2 key chunks -> one 2048-wide exp) plus
  a 2-bank Y tile that alternates between single-chunk scores and the next
  block's Q/K projection psum, giving 3x2048+2x1024 exp batching per head
  while fitting the 8 psum banks alongside the PV accumulators.

Emission order software-pipelines: head h emits its scores/exp interleaved
with PV+normalize of head h-1 and one Q/K projection block, so PE and Act
stay concurrently busy through the 16-head phase.
"""

import numpy as np
import ml_dtypes

B, S, D, H, DH = 4, 1024, 1024, 16, 64
NB = D // 128   # 8 partition blocks
NC2 = D // 256  # 4 pair-chunks for DoubleRow
HW = 65         # head slot width in v1e (64 data + 1 ones col)

_compiled = None


def _build():
    import concourse.bass as bass
    import concourse.mybir as mybir
    import concourse.tile as tile
    from concourse import bacc, masks

    dt = mybir.dt
    f32, bf16, f32r, fp8 = dt.float32, dt.bfloat16, dt.float32r, dt.float8e4
    DR = mybir.MatmulPerfMode.DoubleRow
    ExpF = mybir.ActivationFunctionType.Exp

    nc = bacc.Bacc("TRN2", target_bir_lowering=False, debug=False)

    xqh_d = nc.dram_tensor("xqh", [128, NC2, 2, S], fp8, kind="ExternalInput")
    xqr_d = nc.dram_tensor("xqr", [128, NC2, 2, S], fp8, kind="ExternalInput")
    xkh_d = nc.dram_tensor("xkh", [128, NC2, 2, S], fp8, kind="ExternalInput")
    xkr_d = nc.dram_tensor("xkr", [128, NC2, 2, S], fp8, kind="ExternalInput")
    xvh_d = nc.dram_tensor("xvh", [128, NC2, 2, S], fp8, kind="ExternalInput")
    xvr_d = nc.dram_tensor("xvr", [128, NC2, 2, S], fp8, kind="ExternalInput")
    wqh_d = nc.dram_tensor("wqh", [128, NB, NC2, 2, 128], fp8, kind="ExternalInput")
    wqr_d = nc.dram_tensor("wqr", [128, NB, NC2, 2, 128], fp8, kind="ExternalInput")
    wkh_d = nc.dram_tensor("wkh", [128, NB, NC2, 2, 128], fp8, kind="ExternalInput")
    wkr_d = nc.dram_tensor("wkr", [128, NB, NC2, 2, 128], fp8, kind="ExternalInput")
    wvh_d = nc.dram_tensor("wvh", [128, NC2, 2, D], fp8, kind="ExternalInput")
    wvr_d = nc.dram_tensor("wvr", [128, NC2, 2, D], fp8, kind="ExternalInput")
    wcl_d = nc.dram_tensor("wcl", [128, NB // 2, NB, 128], bf16, kind="ExternalInput")
    wch_d = nc.dram_tensor("wch", [128, NB // 2, NB, 128], bf16, kind="ExternalInput")
    bqkc_d = nc.dram_tensor("bqkc", [128, 3, NB], f32, kind="ExternalInput")
    bvB_d = nc.dram_tensor("bvB", [128, D], bf16, kind="ExternalInput")
    out_d = nc.dram_tensor("outT", [D, S], bf16, kind="ExternalOutput")

    ESCALE = 0.125 / 1024.0  # 1/sqrt(dh) softmax scale / (32*32 weight scale)

    with tile.TileContext(nc) as tc:
        with tc.tile_pool(name="xin", bufs=1) as xip, \
             tc.tile_pool(name="b8", bufs=2) as b8p, \
             tc.tile_pool(name="wv", bufs=1) as wvp, \
             tc.tile_pool(name="pt", bufs=3) as ptp, \
             tc.tile_pool(name="wqk", bufs=2) as wkp, \
             tc.tile_pool(name="qk", bufs=2) as qkp, \
             tc.tile_pool(name="pers", bufs=1) as prp, \
             tc.tile_pool(name="aqp", bufs=2) as aqp, \
             tc.tile_pool(name="rc", bufs=4) as rcp, \
             tc.tile_pool(name="ost", bufs=4) as ostp, \
             tc.tile_pool(name="pxa", bufs=1, space="PSUM") as pxap, \
             tc.tile_pool(name="pxb", bufs=1, space="PSUM") as pxbp, \
             tc.tile_pool(name="pj", bufs=1, space="PSUM") as pjp, \
             tc.tile_pool(name="pv", bufs=1, space="PSUM") as pvp:

            # ---- input DMAs. HWDGE (sync+scalar queues) serializes dispatch
            # at ~650ns/DMA; gpsimd (SWDGE) costs ~1us Pool trigger per DMA
            # but transfers overlap. Q/K-projection inputs lead on both.
            def wblk_load(w_d, m, nm, engine):
                wb = wkp.tile([128, NC2, 2, 128], fp8, tag=nm, name=f"{nm}{m}")
                engine.dma_start(out=wb[:, :, :, :], in_=w_d.ap()[:, m, :, :, :])
                return wb

            xqh_t = xip.tile([128, NC2, 2, S], fp8, tag="xqh")
            nc.gpsimd.dma_start(out=xqh_t[:, :, :, :], in_=xqh_d.ap())
            bqkc_t = xip.tile([128, 3, NB], f32, tag="bqkc")
            nc.sync.dma_start(out=bqkc_t[:, :, :], in_=bqkc_d.ap())
            bq_t, bk_t, bc_t = bqkc_t[:, 0, :], bqkc_t[:, 1, :], bqkc_t[:, 2, :]
            xqr_t = xip.tile([128, NC2, 2, S], fp8, tag="xqr")
            nc.sync.dma_start(out=xqr_t[:, :, :, :], in_=xqr_d.ap())
            wq0h = wblk_load(wqh_d, 0, "wqh", nc.gpsimd)
            wq0r = wblk_load(wqr_d, 0, "wqr", nc.gpsimd)
            wk0h = wblk_load(wkh_d, 0, "wkh", nc.sync)
            wk0r = wblk_load(wkr_d, 0, "wkr", nc.sync)

            xkh_t = xip.tile([128, NC2, 2, S], fp8, tag="xkh")
            nc.sync.dma_start(out=xkh_t[:, :, :, :], in_=xkh_d.ap())
            xkr_t = xip.tile([128, NC2, 2, S], fp8, tag="xkr")
            nc.sync.dma_start(out=xkr_t[:, :, :, :], in_=xkr_d.ap())

            xvh_t = b8p.tile([128, NC2, 2, S], fp8, tag="b8", name="xvh_t")
            nc.sync.dma_start(out=xvh_t[:, :, :, :], in_=xvh_d.ap())
            wvh_t = wvp.tile([128, NC2, 2, D], fp8, tag="wvh")
            nc.gpsimd.dma_start(out=wvh_t[:, :, :, :], in_=wvh_d.ap())
            wvr_t = wvp.tile([128, NC2, 2, D], fp8, tag="wvr")
            nc.sync.dma_start(out=wvr_t[:, :, :, :], in_=wvr_d.ap())
            xvr_t = b8p.tile([128, NC2, 2, S], fp8, tag="b8", name="xvr_t")
            nc.gpsimd.dma_start(out=xvr_t[:, :, :, :], in_=xvr_d.ap())
            bvB_t = xip.tile([128, D], bf16, tag="bv")
            nc.sync.dma_start(out=bvB_t[:, :], in_=bvB_d.ap())

            ident = xip.tile([128, 128], bf16, tag="id")
            masks.make_identity(nc, ident[:, :])

            # persistent tiles
            v1e = prp.tile([128, NB, H * HW], bf16)
            a1 = prp.tile([128, NB, S], bf16, tag="a1")
            ones_ap = v1e[:, :, :].rearrange("p n (h x) -> p n h x", x=HW)[:, :, :, 64]
            nc.vector.memset(ones_ap, 1.0)

            # ---- half-granular projection emitters (fp8 DoubleRow 3-pass) ----
            def vproj_half_mm(ps_view, n2, half):
                first = True
                for xa, wa in ((xvh_t, wvh_t), (xvr_t, wvh_t), (xvh_t, wvr_t)):
                    for c in range(NC2):
                        nc.tensor.matmul(
                            ps_view,
                            xa[:, c, :, n2 * 128:(n2 + 1) * 128],
                            wa[:, c, :, half * 512:(half + 1) * 512],
                            start=first, stop=(wa is wvr_t and c == NC2 - 1),
                            perf_mode=DR,
                        )
                        first = False

            def vproj_drain(ps_view, n2, half):
                dst = v1e[:, n2, :].rearrange(
                    "p (h x) -> p h x", x=HW)[:, half * 8:(half + 1) * 8, 0:64]
                nc.vector.tensor_add(
                    dst,
                    ps_view.rearrange("p (h x) -> p h x", x=64),
                    bvB_t[:, half * 512:(half + 1) * 512].rearrange(
                        "p (h x) -> p h x", x=64))

            def vproj_block(n2, pool):
                ps = pool.tile([128, 2, 512], f32, tag="s", name=f"vps{n2}")
                for half in range(2):
                    vproj_half_mm(ps[:, half, :], n2, half)
                    vproj_drain(ps[:, half, :], n2, half)

            def vproj_half_pj(n2, half):
                ps = pjp.tile([128, 512], f32, tag="s", name=f"vpj{n2}{half}")
                vproj_half_mm(ps[:, :], n2, half)
                vproj_drain(ps[:, :], n2, half)

            def proj_half(wbh, wbr, xh, xr, b_t, ob, m, half):
                ps = pjp.tile([128, 512], f32, tag="s", name=f"pps{m}{half}")
                first = True
                for wa, xa in ((wbh, xh), (wbr, xh), (wbh, xr)):
                    for c in range(NC2):
                        nc.tensor.matmul(
                            ps[:, :], wa[:, c, :, :],
                            xa[:, c, :, half * 512:(half + 1) * 512],
                            start=first, stop=(xa is xr and c == NC2 - 1),
                            perf_mode=DR,
                        )
                        first = False
                nc.vector.tensor_scalar_add(
                    ob[:, half * 512:(half + 1) * 512], ps[:, :], b_t[:, m:m + 1])

            def new_qk(m, which):
                return qkp.tile([128, S], f32r, tag=which, name=f"{which}_{m}")

            # ---- prologue: QK block 0 first (heads start early), then V ----
            q1b = new_qk(0, "q1")
            k1b = new_qk(0, "k1")
            proj_half(wq0h, wq0r, xqh_t, xqr_t, bq_t, q1b, 0, 0)
            proj_half(wk0h, wk0r, xkh_t, xkr_t, bk_t, k1b, 0, 0)
            proj_half(wq0h, wq0r, xqh_t, xqr_t, bq_t, q1b, 0, 1)
            proj_half(wk0h, wk0r, xkh_t, xkr_t, bk_t, k1b, 0, 1)
            for n2 in range(6):
                vproj_block(n2, pxap if n2 % 2 == 0 else pxbp)
            # V blocks 6,7 run through the pj slot during head 0

            # ---- attention phase ----
            state = {}  # deferred work for head h-1

            def scores_group(pt, q1b, k1b, po, g):
                """Half-chunks 3g..3g+2 (g=5: just one) into slot A/B, one
                1536-wide (or 512) exp. Half-chunk j covers scores chunk
                n=j//2, query half j%2 -> pt flat cols [j*512, j*512+512)."""
                pool = pxap if g % 2 == 0 else pxbp
                js = list(range(3 * g, min(3 * g + 3, 16)))
                xs = pool.tile([128, 3, 512], f32, tag="s", name=f"sg{g}")
                for pos, j in enumerate(js):
                    n, half = j // 2, j % 2
                    nc.tensor.matmul(
                        xs[:, pos, :],
                        k1b[po:po + 64, n * 128:(n + 1) * 128],
                        q1b[po:po + 64, half * 512:(half + 1) * 512],
                        start=True, stop=True,
                    )
                flat = pt[:, :, :].rearrange("p a b -> p (a b)")
                np_ = len(js)
                nc.scalar.activation(
                    out=flat[:, 3 * g * 512:(3 * g + np_) * 512],
                    in_=xs[:, 0:np_, :].rearrange("p a b -> p (a b)"),
                    func=ExpF, scale=ESCALE)

            class PVState:
                """PV + normalize for one head; emitted during the next head."""

                def __init__(self, h, pt, aq):
                    self.h, self.pt, self.aq = h, pt, aq
                    self.ps = {}

                def pv(self, tag):
                    g = 0 if tag == "pva" else 1
                    ps = pvp.tile([128, 4, HW], f32, tag="pv",
                                  name=f"pv{self.h}{tag}")
                    self.ps[tag] = ps
                    for qc in range(g * 4, g * 4 + 4):
                        for n in range(NB):
                            nc.tensor.matmul(
                                ps[:, qc - g * 4, :],
                                self.pt[:, n, qc * 128:(qc + 1) * 128],
                                v1e[:, n, self.h * HW:(self.h + 1) * HW],
                                start=(n == 0), stop=(n == NB - 1),
                            )

                def norm(self, tag):
                    g = 0 if tag == "pva" else 1
                    po = (self.h % 2) * 64
                    ps = self.ps[tag]
                    rc = rcp.tile([128, 4, 1], f32, tag="rc",
                                  name=f"rc{self.h}{g}")
                    nc.vector.reciprocal(rc[:, :, :], ps[:, :, 64:65])
                    nc.vector.tensor_mul(
                        self.aq[:, g * 4:(g + 1) * 4, po:po + 64],
                        ps[:, :, 0:64],
                        rc[:, :, :].to_broadcast((128, 4, 64)))

            def transpose_pair(m, aq):
                tp = pvp.tile([128, NB, 128], bf16, tag="pv", name=f"tp{m}")
                for qc in range(NB):
                    nc.tensor.transpose(tp[:, qc, :], aq[:, qc, :], ident[:, :])
                nc.vector.tensor_copy(
                    a1[:, m, :], tp[:, :, :].rearrange("p a b -> p (a b)"))

            qk_tiles = {0: [q1b, k1b]}
            wtiles = {}
            prev = None
            aq_cur = None
            wcl_t = wch_t = None

            # out-projection split: partials (pairs 0-5) pre-run during the
            # proj-free endgame heads; finals (pairs 6,7 + partial add) at the
            # end. Bias is applied in the partial drain.
            part_sb = prp.tile([128, NB, S], bf16, tag="part")
            op_parts = [(m2, hf) for m2 in range(NB) for hf in range(2)]

            def wct_slice(m2, n):
                wct = wcl_t if m2 < NB // 2 else wch_t
                return wct[:, m2 % (NB // 2), n, :]

            def op_partial():
                m2, hf = op_parts.pop(0)
                ps = pjp.tile([128, 512], f32, tag="s", name=f"op{m2}{hf}")
                for n in range(6):
                    nc.tensor.matmul(
                        ps[:, :], wct_slice(m2, n),
                        a1[:, n, hf * 512:(hf + 1) * 512],
                        start=(n == 0), stop=(n == 5),
                    )
                nc.vector.tensor_scalar_add(
                    part_sb[:, m2, hf * 512:(hf + 1) * 512], ps[:, :],
                    bc_t[:, m2:m2 + 1])
            for h in range(H):
                m = h // 2
                po = (h % 2) * 64
                q1b, k1b = qk_tiles[m]
                pt = ptp.tile([128, NB, S], bf16, tag="pt", name=f"pt{h}")
                if h % 2 == 0:
                    aq_cur = aqp.tile([128, NB, 128], bf16, tag="aq", name=f"aq{m}")
                aq_h = aq_cur

                # pj-slot fillers for this head
                fl = []
                if h == 0:
                    wtiles[1] = (wblk_load(wqh_d, 1, "wqh", nc.gpsimd),
                                 wblk_load(wqr_d, 1, "wqr", nc.gpsimd),
                                 wblk_load(wkh_d, 1, "wkh", nc.scalar),
                                 wblk_load(wkr_d, 1, "wkr", nc.scalar))
                    fl = [(vproj_half_pj, (6, 0)), (vproj_half_pj, (6, 1)),
                          (vproj_half_pj, (7, 0)), (vproj_half_pj, (7, 1))]
                elif h == 1:
                    wqhn, wqrn, wkhn, wkrn = wtiles[1]
                    nq, nk = new_qk(1, "q1"), new_qk(1, "k1")
                    qk_tiles[1] = [nq, nk]
                    fl = [(proj_half, (wqhn, wqrn, xqh_t, xqr_t, bq_t, nq, 1, 0)),
                          (proj_half, (wqhn, wqrn, xqh_t, xqr_t, bq_t, nq, 1, 1)),
                          (proj_half, (wkhn, wkrn, xkh_t, xkr_t, bk_t, nk, 1, 0)),
                          (proj_half, (wkhn, wkrn, xkh_t, xkr_t, bk_t, nk, 1, 1))]
                elif m < NB - 1:
                    wqhn, wqrn, wkhn, wkrn = wtiles[m + 1]
                    if h % 2 == 0:
                        nq = new_qk(m + 1, "q1")
                        qk_tiles[m + 1] = [nq, None]
                        fl = [(proj_half, (wqhn, wqrn, xqh_t, xqr_t, bq_t, nq,
                                           m + 1, hf)) for hf in range(2)]
                    else:
                        nk = new_qk(m + 1, "k1")
                        qk_tiles[m + 1][1] = nk
                        fl = [(proj_half, (wkhn, wkrn, xkh_t, xkr_t, bk_t, nk,
                                           m + 1, hf)) for hf in range(2)]

                # stream weights two blocks ahead (odd heads)
                if h % 2 == 1 and m + 2 < NB:
                    wtiles[m + 2] = (wblk_load(wqh_d, m + 2, "wqh", nc.sync),
                                     wblk_load(wqr_d, m + 2, "wqr", nc.sync),
                                     wblk_load(wkh_d, m + 2, "wkh", nc.gpsimd),
                                     wblk_load(wkr_d, m + 2, "wkr", nc.gpsimd))
                if h == 1:
                    # wc loads reuse the xvh/xvr slots (b8 ring); V-proj done
                    wcl_t = b8p.tile([128, NB // 2, NB, 128], bf16, tag="b8",
                                     name="wcl_t")
                    nc.scalar.dma_start(out=wcl_t[:, :, :, :], in_=wcl_d.ap())
                    wch_t = b8p.tile([128, NB // 2, NB, 128], bf16, tag="b8",
                                     name="wch_t")
                    nc.scalar.dma_start(out=wch_t[:, :, :, :], in_=wch_d.ap())

                if h == 0:
                    # custom: V6/V7 pj fillers interleaved between groups
                    for g in range(6):
                        scores_group(pt, q1b, k1b, po, g)
                        if fl and g % 2 == 0:
                            f, a = fl.pop(0)
                            f(*a)
                        if fl and g == 5:
                            f, a = fl.pop(0)
                            f(*a)
                else:
                    scores_group(pt, q1b, k1b, po, 0)
                    scores_group(pt, q1b, k1b, po, 1)
                    # transposes of the pair finished two heads ago: emitted
                    # here so they execute once its norms drain, while PE has
                    # scores to chew on
                    if h >= 3 and h % 2 == 1:
                        transpose_pair(h // 2 - 1, aq_prev)
                    if prev is not None:
                        prev.pv("pva")
                        prev.norm("pva")
                    def filler():
                        if fl:
                            f, a = fl.pop(0)
                            f(*a)
                        elif h >= 14 and op_parts:
                            op_partial()

                    scores_group(pt, q1b, k1b, po, 2)
                    filler()
                    scores_group(pt, q1b, k1b, po, 3)
                    filler()
                    scores_group(pt, q1b, k1b, po, 4)
                    scores_group(pt, q1b, k1b, po, 5)
                    if prev is not None:
                        prev.pv("pvb")
                        prev.norm("pvb")
                    while fl:
                        f, a = fl.pop(0)
                        f(*a)
                    if h == 13 and op_parts:
                        op_partial()
                    if h >= 14:
                        for _ in range(2):
                            if op_parts:
                                op_partial()

                if h % 2 == 1:
                    aq_prev = aq_h
                prev = PVState(h, pt, aq_h)

            # finish head 15
            prev.pv("pva")
            prev.norm("pva")
            prev.pv("pvb")
            prev.norm("pvb")
            transpose_pair(7, prev.aq)

            # ---- out-projection finals: pairs 6,7 + partial add; remaining
            # partials (pj) interleave with finals (other rings) ----
            queues = [nc.sync, nc.gpsimd, nc.scalar]
            opools = [(pxap, "s"), (pxbp, "s"), (pvp, "pv")]
            for i, (m2, hf) in enumerate(
                    [(m2, hf) for m2 in range(NB) for hf in range(2)]):
                if op_parts:
                    op_partial()
                pool, ptag = opools[i % 3]
                ps = pool.tile([128, 512], f32, tag=ptag, name=f"of{m2}{hf}")
                for n in (6, 7):
                    nc.tensor.matmul(
                        ps[:, :], wct_slice(m2, n),
                        a1[:, n, hf * 512:(hf + 1) * 512],
                        start=(n == 6), stop=(n == 7),
                    )
                ot = ostp.tile([128, 512], bf16, tag="ost", name=f"ot{m2}{hf}")
                nc.vector.tensor_add(ot[:, :], ps[:, :],
                                     part_sb[:, m2, hf * 512:(hf + 1) * 512])
                queues[i % 3].dma_start(
                    out=out_d.ap()[m2 * 128:(m2 + 1) * 128,
                                   hf * 512:(hf + 1) * 512],
                    in_=ot[:, :])

    nc.compile()
    return nc


def _get_nc():
    global _compiled
    if _compiled is None:
        _compiled = _build()
    return _compiled


def _fp8_split(a):
    e4m3 = ml_dtypes.float8_e4m3
    h = np.ascontiguousarray(a).astype(e4m3)
    r = (a - h.astype(np.float32)).astype(e4m3)
    return h, r


def _make_in_maps(q, k, v, Wq, bq, Wk, bk, Wv, bv, Wq2, bq2, Wk2, bk2, Wv2, bv2,
                  Wc, bc, Wc2, bc2):
    bf16 = ml_dtypes.bfloat16

    def xpack(x):  # [s, d] -> 2x [128, c, 2, s] fp8 (d = (2c+j)*128+p)
        xt = np.asarray(x, np.float32).T.reshape(NC2, 2, 128, S).transpose(2, 0, 1, 3)
        return _fp8_split(np.ascontiguousarray(xt))

    def wqkpack(w):  # W[e,d]*32 -> 2x [128 p, m, c, 2, e'] fp8
        wt = (32.0 * np.asarray(w, np.float32)).reshape(
            NB, 128, NC2, 2, 128).transpose(4, 0, 2, 3, 1)
        return _fp8_split(np.ascontiguousarray(wt))

    def wvpack(w):  # Wv[e,d]*32 -> 2x [128 p, c, 2, e] fp8
        wt = (32.0 * np.asarray(w, np.float32)).T.reshape(
            NC2, 2, 128, D).transpose(2, 0, 1, 3)
        return _fp8_split(np.ascontiguousarray(wt))

    def wcpack(w):  # Wc[e,d]/32 -> [128 p, m, n, e'] bf16, split in m halves
        ws = (np.asarray(w, np.float32) / 32.0).reshape(
            NB, 128, NB, 128).transpose(3, 0, 2, 1)
        ws = np.ascontiguousarray(ws).astype(bf16)
        return ws[:, :NB // 2], ws[:, NB // 2:]

    def btile(b, scale):
        return np.ascontiguousarray(
            (scale * np.asarray(b, np.float32)).reshape(NB, 128).T)

    def brep(b, scale):
        return np.ascontiguousarray(np.broadcast_to(
            scale * np.asarray(b, np.float32), (128, D))).astype(bf16)

    paths = []
    for (Wq_, bq_, Wk_, bk_, Wv_, bv_, Wc_, bc_) in (
            (Wq, bq, Wk, bk, Wv, bv, Wc, bc),
            (Wq2, bq2, Wk2, bk2, Wv2, bv2, Wc2, bc2)):
        wqh, wqr = wqkpack(Wq_)
        wkh, wkr = wqkpack(Wk_)
        wvh, wvr = wvpack(Wv_)
        wcl, wch = wcpack(Wc_)
        bqkc = np.ascontiguousarray(np.stack(
            [btile(bq_, 32.0), btile(bk_, 32.0), btile(bc_, 1.0)], axis=1))
        paths.append(dict(
            wqh=wqh, wqr=wqr, wkh=wkh, wkr=wkr, wvh=wvh, wvr=wvr,
            wcl=wcl, wch=wch, bqkc=bqkc, bvB=brep(bv_, 32.0)))

    xs = {}
    for nmx, arr in (("q", q), ("k", k), ("v", v)):
        for b in range(B):
            xs[(nmx, b)] = xpack(arr[b])

    in_maps = []
    for c in range(8):
        p, b = c // 4, c % 4
        if p == 0:
            (xqh, xqr), (xkh, xkr), (xvh, xvr) = xs[("q", b)], xs[("k", b)], xs[("v", b)]
        else:
            # path 2: q2 from k; k2, v2 from q
            (xqh, xqr), (xkh, xkr), (xvh, xvr) = xs[("k", b)], xs[("q", b)], xs[("q", b)]
        in_maps.append(dict(paths[p], xqh=xqh, xqr=xqr, xkh=xkh, xkr=xkr,
                            xvh=xvh, xvr=xvr))
    return in_maps


def _run(in_maps, trace=False):
    from concourse.bass_utils import run_bass_kernel_spmd
    nc = _get_nc()
    return run_bass_kernel_spmd(nc, in_maps, core_ids=list(range(8)), trace=trace)


def kernel(**inputs):
    in_maps = _make_in_maps(**inputs)
    try:
        res = _run(in_maps)
    except Exception:
        # transient NRT_EXEC_UNIT_UNRECOVERABLE has been observed when a
        # prior process crashed mid-execution; one retry reloads the NEFF
        res = _run(in_maps)
    out1 = np.stack([res.results[b]["outT"].T for b in range(4)]).astype(np.float32)
    out2 = np.stack([res.results[4 + b]["outT"].T for b in range(4)]).astype(np.float32)
    return out1, out2
